# revision 28
# baseline (speedup 1.0000x reference)
"""Trainium2 Bass kernel for nn_BlocksCore (RIMs-style BlocksCore forward).

Data-parallel over batch: 8 cores x 2048 tokens. Feature-major activations.
Key design points vs the naive version:
  - all wide matmuls use f32r or bf16 moving operands (1 cyc/row);
  - sigmoid-free: every sigmoid is tanh-based (sigma(x)=0.5+0.5*tanh(x/2)),
    folded into selection matmuls where linear, so the Act engine only ever
    needs the {tanh, exp, copy} table set (zero activation-table reloads);
  - softmax normalization batched over 3-block groups;
  - final masked mix via tensor_copy + copy_predicated (exact passthrough
    semantics up to bf16 output rounding);
  - elementwise work spread across DVE / Pool / Act.
"""
import sys
sys.path.insert(0, '/opt/trn_rl_repo')
import numpy as np
import concourse.bacc as bacc
import concourse.mybir as mybir
from concourse.tile import TileContext
from concourse.bass_utils import run_bass_kernel_spmd

NINP, NHID, K, TOPK = 768, 360, 6, 4
BS = NHID // K
NH_IN, DK_IN = 4, 64
NH_C, DK_C, DV_C = 4, 32, 32
B = 16384
NCORES = 8
NLOC = B // NCORES
NT = 512
NTILES = NLOC // NT

F32, F32R, BF16 = mybir.dt.float32, mybir.dt.float32r, mybir.dt.bfloat16
AF = mybir.ActivationFunctionType
OP = mybir.AluOpType

# weight name -> on-chip dtype
WDT = {
    "Wk1": F32R, "Wq": F32R, "Wv1": F32R, "sel_s1": F32R, "sel_iatt": F32R,
    "ident": F32,
    "E_bc": BF16, "A": BF16, "Whh": F32R, "bias": F32,
    "Wqc": BF16, "Wkc": BF16, "Wvc": BF16,
    "sel_sc": BF16, "sel_z": BF16, "sel_z4": BF16, "sel_hb": BF16,
    "Wfg": BF16, "fgb": F32, "E_mask": BF16,
}
_CACHE = {}


def _build(wshapes):
    nc = bacc.Bacc("TRN2", target_bir_lowering=False, debug=False)

    d_inp = nc.dram_tensor("inpT", [128, 6, NLOC], F32R, kind="ExternalInput")
    d_hx_bm = nc.dram_tensor("hx_bm", [60, 6, NLOC], F32R, kind="ExternalInput")
    d_cx_bm = nc.dram_tensor("cx_bm", [60, 6, NLOC], BF16, kind="ExternalInput")
    dW = {n: nc.dram_tensor(n, list(s), WDT[n], kind="ExternalInput")
          for n, s in wshapes.items()}
    d_hxo = nc.dram_tensor("hxo", [60, 6, NLOC], BF16, kind="ExternalOutput")
    d_cxo = nc.dram_tensor("cxo", [60, 6, NLOC], BF16, kind="ExternalOutput")

    with TileContext(nc) as tc:
        with tc.tile_pool(name="wp", bufs=1) as wp, \
             tc.tile_pool(name="io", bufs=2) as io, \
             tc.tile_pool(name="s1", bufs=1) as s1p, \
             tc.tile_pool(name="s2", bufs=2) as s2p, \
             tc.tile_pool(name="ppE", bufs=2, space="PSUM") as ppE, \
             tc.tile_pool(name="ppL", bufs=2, space="PSUM") as ppL, \
             tc.tile_pool(name="ppS", bufs=1, space="PSUM") as ppS, \
             tc.tile_pool(name="ppF", bufs=1, space="PSUM") as ppF:

            W = {}
            wengs = [nc.sync, nc.gpsimd, nc.scalar]
            for wi, (n, s) in enumerate(wshapes.items()):
                W[n] = wp.tile(list(s), WDT[n], tag=n, name=n)
                wengs[wi % 3].dma_start(out=W[n], in_=dW[n].ap())

            for ti in range(NTILES):
                t0 = ti * NT
                # ---------------- DMA in ----------------
                inp_r = io.tile([128, 6, NT], F32R, tag="inp", bufs=1)
                nc.sync.dma_start(out=inp_r, in_=d_inp.ap()[:, :, t0:t0 + NT])
                hxf = io.tile([60, 6, NT], F32R, tag="hxf")
                nc.sync.dma_start(out=hxf, in_=d_hx_bm.ap()[:, :, t0:t0 + NT])
                cxb = io.tile([60, 6, NT], BF16, tag="cxb")
                nc.sync.dma_start(out=cxb, in_=d_cx_bm.ap()[:, :, t0:t0 + NT])

                # ---------------- input attention ----------------
                psK1 = ppE.tile([128, 2 * NT], F32, tag="e2")
                for m in range(2):
                    for c in range(6):
                        nc.tensor.matmul(psK1[:, m * NT:(m + 1) * NT],
                                         lhsT=W["Wk1"][:, c, m * 128:(m + 1) * 128],
                                         rhs=inp_r[:, c, :],
                                         start=(c == 0), stop=(c == 5))
                k1 = s1p.tile([128, 2 * NT], F32R, tag="k1")
                nc.scalar.copy(out=k1, in_=psK1)

                psV1 = ppE.tile([120, 2 * NT], F32, tag="e2")
                for m in range(2):
                    for c in range(6):
                        nc.tensor.matmul(psV1[:, m * NT:(m + 1) * NT],
                                         lhsT=W["Wv1"][:, c, m * 120:(m + 1) * 120],
                                         rhs=inp_r[:, c, :],
                                         start=(c == 0), stop=(c == 5))
                v1b = s1p.tile([120, 2 * NT], BF16, tag="v1b")
                nc.scalar.copy(out=v1b, in_=psV1)

                psS1 = ppE.tile([32, NT], F32, tag="e2")
                for i in range(K):
                    psQ = ppE.tile([128, 2 * NT], F32, tag="e2")
                    for m in range(2):
                        nc.tensor.matmul(psQ[:, m * NT:(m + 1) * NT],
                                         lhsT=W["Wq"][:, i, m * 128:(m + 1) * 128],
                                         rhs=hxf[:, i, :], start=True, stop=True)
                    P = s2p.tile([128, 2 * NT], F32R, tag="P", bufs=1)
                    nc.vector.tensor_mul(out=P, in0=psQ, in1=k1)
                    for c in range(2):
                        nc.tensor.matmul(psS1,
                                         lhsT=W["sel_s1"][:, i * 2 + c, :],
                                         rhs=P[:, c * NT:(c + 1) * NT],
                                         start=(i == 0 and c == 0),
                                         stop=(i == 5 and c == 1))
                # sigt rows 0:24 = tanh(s/2); row 32 = 1.0 (for sigma fold)
                sigt = s1p.tile([64, NT], BF16, tag="sigt")
                nc.scalar.activation(out=sigt[0:32, :], in_=psS1[0:32, :],
                                     func=AF.Tanh, scale=0.5)
                nc.vector.memset(sigt[32:33, :], 1.0)
                # negt = tanh(-s/2): rank-equivalent to mean sigmoid(-s)
                negt = s1p.tile([24, NT], F32R, tag="negt")
                nc.scalar.activation(out=negt, in_=psS1[0:24, :], func=AF.Tanh,
                                     scale=-0.5)
                psIatt = ppL.tile([32, NT], F32, tag="l1")
                nc.tensor.matmul(psIatt, lhsT=W["sel_iatt"], rhs=negt,
                                 start=True, stop=True)
                iatt = s1p.tile([6, NT], F32, tag="iatt")
                nc.scalar.copy(out=iatt, in_=psIatt[0:6, :])

                # ---- top-2-of-null-attention mask (token-major) ----
                maskT = s1p.tile([128, 4 * 6], F32, tag="maskT")
                for c in range(4):
                    psIT = ppL.tile([128, 8], F32, tag="l1")
                    nc.tensor.transpose(psIT[:, 0:6], iatt[:, c * 128:(c + 1) * 128],
                                        W["ident"][0:6, 0:6])
                    it8 = s1p.tile([128, 8], F32, tag="it8")
                    nc.vector.memset(it8[:, 6:8], -1e30)
                    nc.vector.tensor_copy(it8[:, 0:6], psIT[:, 0:6])
                    mx = s1p.tile([128, 8], F32, tag="mx")
                    nc.vector.max(out=mx, in_=it8)
                    nc.vector.tensor_scalar(maskT[:, c * 6:(c + 1) * 6],
                                            it8[:, 0:6], mx[:, 1:2],
                                            scalar2=None, op0=OP.is_lt)
                psMask = ppL.tile([6, NT], F32, tag="l1")
                for c in range(4):
                    nc.tensor.transpose(psMask[:, c * 128:(c + 1) * 128],
                                        maskT[:, c * 6:(c + 1) * 6], W["ident"])
                mask6 = s1p.tile([6, NT], BF16, tag="mask6")
                nc.scalar.copy(out=mask6, in_=psMask)

                # ---------------- att_in + LSTM ----------------
                sI = s1p.tile([60, 6 * NT], BF16, tag="sI")
                sF = s1p.tile([60, 6 * NT], BF16, tag="sF")
                sO = s1p.tile([60, 6 * NT], BF16, tag="sO")
                tg = s1p.tile([60, 6 * NT], BF16, tag="tg")
                for i in range(K):
                    psBc = ppE.tile([120, 2 * NT], F32, tag="e2")
                    for m in range(2):
                        nc.tensor.matmul(psBc[:, m * NT:(m + 1) * NT],
                                         lhsT=W["E_bc"][:, i, m * 120:(m + 1) * 120],
                                         rhs=sigt[0:33, :],
                                         start=True, stop=True)
                    attin0 = s2p.tile([120, NT], BF16, tag="attin")
                    attin1 = s2p.tile([120, NT], BF16, tag="attin")
                    nc.vector.tensor_mul(out=attin0, in0=psBc[:, 0:NT],
                                         in1=v1b[:, 0:NT])
                    nc.vector.tensor_mul(out=attin1, in0=psBc[:, NT:2 * NT],
                                         in1=v1b[:, NT:2 * NT])
                    attins = (attin0, attin1)
                    psG = ppE.tile([128, 2 * NT], F32, tag="e2")
                    for m in range(2):
                        for c in range(2):
                            nc.tensor.matmul(psG[:, m * NT:(m + 1) * NT],
                                             lhsT=W["A"][:, i * 2 + c, m * 128:(m + 1) * 128],
                                             rhs=attins[c],
                                             start=(c == 0), stop=False)
                        nc.tensor.matmul(psG[:, m * NT:(m + 1) * NT],
                                         lhsT=W["Whh"][:, i, m * 128:(m + 1) * 128],
                                         rhs=hxf[:, i, :], start=False, stop=True)
                    # tanh-form gates (Act), then in-place affine to sigma
                    sl_i = slice(i * NT, (i + 1) * NT)
                    nc.scalar.activation(out=sI[:, sl_i], in_=psG[0:60, 0:NT],
                                         func=AF.Tanh, scale=0.5,
                                         bias=W["bias"][0:60, 2 * i:2 * i + 1])
                    nc.scalar.activation(out=sF[:, sl_i], in_=psG[64:124, 0:NT],
                                         func=AF.Tanh, scale=0.5,
                                         bias=W["bias"][64:124, 2 * i:2 * i + 1])
                    nc.scalar.activation(out=sO[:, sl_i], in_=psG[0:60, NT:2 * NT],
                                         func=AF.Tanh, scale=0.5,
                                         bias=W["bias"][0:60, 2 * i + 1:2 * i + 2])
                    nc.scalar.activation(out=tg[:, sl_i],
                                         in_=psG[64:124, NT:2 * NT], func=AF.Tanh,
                                         bias=W["bias"][64:124, 2 * i + 1:2 * i + 2])
                    nc.gpsimd.tensor_scalar(sI[:, sl_i], sI[:, sl_i], 0.5, 0.5,
                                            op0=OP.mult, op1=OP.add)
                    nc.gpsimd.tensor_scalar(sF[:, sl_i], sF[:, sl_i], 0.5, 0.5,
                                            op0=OP.mult, op1=OP.add)
                    nc.gpsimd.tensor_scalar(sO[:, sl_i], sO[:, sl_i], 0.5, 0.5,
                                            op0=OP.mult, op1=OP.add)
                cxb2 = cxb.rearrange("p c t -> p (c t)")
                t1 = s1p.tile([60, 6 * NT], BF16, tag="t1")
                t2 = s1p.tile([60, 6 * NT], BF16, tag="t2")
                cnew = s1p.tile([60, 6 * NT], BF16, tag="cnew", bufs=2)
                hxnew = s1p.tile([60, 6 * NT], BF16, tag="hxnew")
                for q in range(3):
                    sl = slice(2 * q * NT, 2 * (q + 1) * NT)
                    nc.vector.tensor_mul(out=t1[:, sl], in0=sF[:, sl],
                                         in1=cxb2[:, sl])
                    nc.gpsimd.tensor_mul(out=t2[:, sl], in0=sI[:, sl],
                                         in1=tg[:, sl])
                    nc.vector.tensor_add(out=cnew[:, sl], in0=t1[:, sl],
                                         in1=t2[:, sl])
                    nc.scalar.activation(out=t1[:, sl], in_=cnew[:, sl],
                                         func=AF.Tanh)
                    nc.gpsimd.tensor_mul(out=hxnew[:, sl], in0=sO[:, sl],
                                         in1=t1[:, sl])

                # ---------------- communication attention ----------------
                QKc = s1p.tile([128, 2, 6, NT], BF16, tag="QKc")
                VC = s1p.tile([128, 6, NT], BF16, tag="VC")
                for i in range(K):
                    psQc = ppL.tile([128, NT], F32, tag="l1")
                    nc.tensor.matmul(psQc, lhsT=W["Wqc"][:, i, :],
                                     rhs=hxnew[:, i * NT:(i + 1) * NT],
                                     start=True, stop=True)
                    nc.scalar.copy(out=QKc[:, 0, i, :], in_=psQc)
                    psKc = ppL.tile([128, NT], F32, tag="l1")
                    nc.tensor.matmul(psKc, lhsT=W["Wkc"][:, i, :],
                                     rhs=hxnew[:, i * NT:(i + 1) * NT],
                                     start=True, stop=True)
                    nc.scalar.copy(out=QKc[:, 1, i, :], in_=psKc)
                    psV = ppL.tile([128, NT], F32, tag="l1")
                    nc.tensor.matmul(psV, lhsT=W["Wvc"][:, i, :],
                                     rhs=hxnew[:, i * NT:(i + 1) * NT],
                                     start=True, stop=True)
                    nc.scalar.copy(out=VC[:, i, :], in_=psV)

                attC = s1p.tile([60, 6 * NT], BF16, tag="t2")
                for i in range(K):
                    psSc = ppS.tile([32, NT], F32, tag="accS")
                    for half in range(2):
                        Pc = s2p.tile([128, 3, NT], BF16, tag="Pc", bufs=1)
                        nc.gpsimd.tensor_mul(
                            out=Pc,
                            in0=QKc[:, 1, 3 * half:3 * half + 3, :],
                            in1=QKc[:, 0, i, :]
                                .rearrange("p (j t) -> p j t", j=1)
                                .broadcast_to([128, 3, NT]))
                        for jj in range(3):
                            j = 3 * half + jj
                            nc.tensor.matmul(psSc, lhsT=W["sel_sc"][:, j, :],
                                             rhs=Pc[:, jj, :],
                                             start=(j == 0), stop=(j == 5))
                    expS = s2p.tile([32, NT], BF16, tag="expS", bufs=2)
                    nc.scalar.activation(out=expS, in_=psSc[0:32, :], func=AF.Exp)
                    psZ = ppL.tile([4, NT], F32, tag="l1")
                    nc.tensor.matmul(psZ, lhsT=W["sel_z"], rhs=expS,
                                     start=True, stop=True)
                    rz = s2p.tile([4, NT], BF16, tag="rz", bufs=1)
                    with nc.allow_low_precision("attn normalization in bf16 is fine"):
                        nc.vector.reciprocal(out=rz, in_=psZ)
                    psRZ = ppL.tile([24, NT], F32, tag="l1")
                    nc.tensor.matmul(psRZ, lhsT=W["sel_z4"], rhs=rz,
                                     start=True, stop=True)
                    attn = s2p.tile([24, NT], BF16, tag="attn_i", bufs=2)
                    nc.vector.tensor_mul(out=attn, in0=expS[0:24, :], in1=psRZ)
                    psFG = ppF.tile([124, NT], F32, tag="accF")
                    for j in range(K):
                        psAb = ppL.tile([128, NT], F32, tag="l1")
                        nc.tensor.matmul(psAb,
                                         lhsT=W["sel_hb"][:, j, :],
                                         rhs=attn,
                                         start=True, stop=True)
                        Pav = s2p.tile([128, NT], BF16, tag="Pav", bufs=2)
                        nc.vector.tensor_mul(out=Pav, in0=psAb,
                                             in1=VC[:, j, :])
                        nc.tensor.matmul(psFG, lhsT=W["Wfg"],
                                         rhs=Pav,
                                         start=(j == 0),
                                         stop=(j == 5))
                    tf_ = s2p.tile([60, NT], BF16, tag="tf_", bufs=1)
                    nc.scalar.activation(out=tf_, in_=psFG[0:60, :], func=AF.Tanh,
                                         bias=W["fgb"][0:60, 0:1])
                    t2g = s2p.tile([60, NT], BF16, tag="t2g", bufs=1)
                    nc.scalar.activation(out=t2g, in_=psFG[64:124, :],
                                         func=AF.Tanh, scale=0.5,
                                         bias=W["fgb"][64:124, 0:1])
                    sg_ = s2p.tile([60, NT], BF16, tag="sg_", bufs=1)
                    nc.gpsimd.tensor_scalar(sg_, t2g, 0.5, 0.5,
                                            op0=OP.mult, op1=OP.add)
                    nc.gpsimd.tensor_mul(out=attC[:, i * NT:(i + 1) * NT],
                                         in0=sg_, in1=tf_)

                for q in range(3):
                    sl = slice(2 * q * NT, 2 * (q + 1) * NT)
                    nc.gpsimd.tensor_add(out=hxnew[:, sl], in0=hxnew[:, sl],
                                         in1=attC[:, sl])

                # ---------------- masked output mix ----------------
                hxo_t = io.tile([60, 6, NT], BF16, tag="hxo_t", bufs=1)
                cxo_t = io.tile([60, 6, NT], BF16, tag="cxo_t", bufs=1)
                hxf2 = hxf.rearrange("p c t -> p (c t)")
                hxo2 = hxo_t.rearrange("p c t -> p (c t)")
                cxo2 = cxo_t.rearrange("p c t -> p (c t)")
                cxb3 = cxb.rearrange("p c t -> p (c t)")
                for q in range(3):
                    slq = slice(2 * q * NT, 2 * (q + 1) * NT)
                    nc.gpsimd.tensor_copy(hxo2[:, slq], hxf2[:, slq])
                    nc.vector.tensor_copy(cxo2[:, slq], cxb3[:, slq])
                for i in range(K):
                    psMb = ppL.tile([60, NT], F32, tag="l1")
                    nc.tensor.matmul(psMb,
                                     lhsT=W["E_mask"][:, i, :], rhs=mask6,
                                     start=True, stop=True)
                    sl = slice(i * NT, (i + 1) * NT)
                    nc.vector.copy_predicated(hxo2[:, sl], psMb, hxnew[:, sl])
                    nc.vector.copy_predicated(cxo2[:, sl], psMb, cnew[:, sl])
                nc.sync.dma_start(out=d_hxo.ap()[:, :, t0:t0 + NT], in_=hxo_t)
                nc.sync.dma_start(out=d_cxo.ap()[:, :, t0:t0 + NT], in_=cxo_t)

    nc.compile()
    return nc


def _prep_weights(inputs):
    f32 = np.float32
    Wq_inp = np.asarray(inputs['Wq_inp'], f32)
    Wk_inp = np.asarray(inputs['Wk_inp'], f32)
    Wv_inp = np.asarray(inputs['Wv_inp'], f32)
    W_ih = np.asarray(inputs['W_ih'], f32)
    W_hh = np.asarray(inputs['W_hh'], f32)
    bsum = (np.asarray(inputs['b_ih'], f32) + np.asarray(inputs['b_hh'], f32))
    Wq_c = np.asarray(inputs['Wq_c'], f32)
    Wk_c = np.asarray(inputs['Wk_c'], f32)
    Wv_c = np.asarray(inputs['Wv_c'], f32)
    fc_w = np.asarray(inputs['fc_w'], f32)
    gate_w = np.asarray(inputs['gate_w'], f32)
    fc_b = np.asarray(inputs['fc_b'], f32)
    gate_b = np.asarray(inputs['gate_b'], f32)

    w = {}
    w["Wk1"] = (Wk_inp[1] / np.sqrt(DK_IN)).reshape(6, 128, 256).transpose(1, 0, 2)
    w["Wq"] = Wq_inp.transpose(1, 0, 2)                       # [60, 6, 256]
    w["Wv1"] = Wv_inp[1].reshape(6, 128, 240).transpose(1, 0, 2)
    sel = np.zeros((128, 12, 32), f32)
    for i in range(K):
        for c in range(2):
            for hh in range(2):
                h = c * 2 + hh
                sel[hh * 64:(hh + 1) * 64, i * 2 + c, i * 4 + h] = 1.0
    w["sel_s1"] = sel
    si = np.zeros((24, 32), f32)
    for i in range(K):
        si[i * 4:(i + 1) * 4, i] = 0.25
    w["sel_iatt"] = si
    w["ident"] = np.eye(128, dtype=f32)
    # E_bc with sigma fold: psBc = 0.5*tanh_bcast + 0.5 (row 32 is constant 1)
    Eb = np.zeros((33, 6, 240), f32)
    for i in range(K):
        for h in range(4):
            Eb[i * 4 + h, i, h * 60:(h + 1) * 60] = 0.5
    Eb[32, :, :] = 0.5
    w["E_bc"] = Eb
    A = np.zeros((120, 12, 256), f32)
    Whh_l = np.zeros((60, 6, 256), f32)
    bias = np.zeros((128, 12), f32)
    # gate column layout per 128-col half: half0=[gi@0, gf@64], half1=[go@0, gg@64]
    gate_pos = {0: (0, 0), 1: (0, 64), 3: (1, 0), 2: (1, 64)}  # g -> (half, col)
    # bias: tanh-form gates need bias*0.5 for i,f,o (scale 0.5); full for g
    bias_scale = {0: 0.5, 1: 0.5, 3: 0.5, 2: 1.0}
    for i in range(K):
        for g in range(4):
            half, co = gate_pos[g]
            wb = W_ih[g * NHID + i * BS:g * NHID + (i + 1) * BS,
                      i * 240:(i + 1) * 240]             # [60 gate rows, 240 att]
            for c in range(2):
                A[:, i * 2 + c, half * 128 + co:half * 128 + co + 60] = \
                    wb[:, c * 120:(c + 1) * 120].T
            hh = W_hh[g * NHID + i * BS:g * NHID + (i + 1) * BS,
                      i * BS:(i + 1) * BS]               # [60, 60]
            Whh_l[:, i, half * 128 + co:half * 128 + co + 60] = hh.T
            bias[co:co + 60, 2 * i + half] = \
                bias_scale[g] * bsum[g * NHID + i * BS:g * NHID + (i + 1) * BS]
    w["A"] = A
    w["Whh"] = Whh_l
    w["bias"] = bias
    w["Wqc"] = (Wq_c / np.sqrt(DK_C)).transpose(1, 0, 2)
    w["Wkc"] = Wk_c.transpose(1, 0, 2)
    w["Wvc"] = Wv_c.transpose(1, 0, 2)
    ss = np.zeros((128, 6, 32), f32)
    for j in range(K):
        for h in range(4):
            ss[h * 32:(h + 1) * 32, j, j * 4 + h] = 1.0
    w["sel_sc"] = ss
    sz = np.zeros((32, 4), f32)
    for j in range(K):
        for h in range(4):
            sz[j * 4 + h, h] = 1.0
    w["sel_z"] = sz
    sz4 = np.zeros((4, 24), f32)
    for j in range(K):
        for h in range(4):
            sz4[h, j * 4 + h] = 1.0
    w["sel_z4"] = sz4
    shb = np.zeros((24, 6, 128), f32)
    for j in range(K):
        for h in range(4):
            shb[j * 4 + h, j, h * 32:(h + 1) * 32] = 1.0
    w["sel_hb"] = shb
    Wfg = np.zeros((128, 124), f32)
    Wfg[:, 0:60] = fc_w.T
    Wfg[:, 64:124] = gate_w.T
    w["Wfg"] = Wfg
    fgb = np.zeros((128, 1), f32)
    fgb[0:60, 0] = fc_b
    fgb[64:124, 0] = 0.5 * gate_b    # t2g uses scale 0.5
    w["fgb"] = fgb
    Em = np.zeros((6, 6, 60), f32)
    for i in range(K):
        Em[i, i, :] = 1.0
    w["E_mask"] = Em
    return {k: np.ascontiguousarray(v, f32) for k, v in w.items()}


def kernel(**inputs):
    idx = int(np.asarray(inputs['idx_layer']))
    inp = np.asarray(inputs['inp'], np.float32)
    hx = np.asarray(inputs['hx'], np.float32)[idx]
    cx = np.asarray(inputs['cx'], np.float32)[idx]

    w = _prep_weights(inputs)
    if "built" not in _CACHE:
        _CACHE["built"] = _build({k: v.shape for k, v in w.items()})
    nc = _CACHE["built"]

    inpT = inp.T.reshape(6, 128, B).transpose(1, 0, 2)
    hx_bm = hx.T.reshape(6, 60, B).transpose(1, 0, 2)
    cx_bm = cx.T.reshape(6, 60, B).transpose(1, 0, 2)

    bf16 = mybir.dt.np(BF16)
    wconv = {n: (np.ascontiguousarray(v.astype(bf16)) if WDT[n] == BF16 else v)
             for n, v in w.items()}
    cx_b = cx_bm.astype(bf16)
    in_maps = []
    for c in range(NCORES):
        sl = slice(c * NLOC, (c + 1) * NLOC)
        m = {"inpT": np.ascontiguousarray(inpT[:, :, sl]),
             "hx_bm": np.ascontiguousarray(hx_bm[:, :, sl]),
             "cx_bm": np.ascontiguousarray(cx_b[:, :, sl])}
        m.update(wconv)
        in_maps.append(m)

    res = run_bass_kernel_spmd(nc, in_maps, core_ids=list(range(NCORES)))
    hxo = np.concatenate(
        [np.asarray(r["hxo"], np.float32).transpose(1, 0, 2).reshape(NHID, NLOC).T
         for r in res.results], axis=0)
    cxo = np.concatenate(
        [np.asarray(r["cxo"], np.float32).transpose(1, 0, 2).reshape(NHID, NLOC).T
         for r in res.results], axis=0)
    return np.asarray(hxo, np.float32), np.asarray(cxo, np.float32)
